# revision 9
# baseline (speedup 1.0000x reference)
"""Trainium2 Bass kernel: ConvNeXt MLP + parallel MoE-LoRA (data-parallel over tokens).

Math per token t (D=512, Dh=2048, E=3 experts, r=8, top-k=2):
    base = gelu(x @ W1 + b1) @ W2 + b2
    g_e  = gelu(x @ w_down[e]) * wts[e, t]          (wts from top-k routing)
    out  = base + sum_e g_e @ w_up[e]

Strategy (per NeuronCore, 8 cores data-parallel on the token dim):
  - weights arrive pre-packed bf16 from host (chunked layouts ready for
    direct DMA into SBUF) -- no on-device weight staging/casting.
  - tokens tiled 128 at a time; supergroups of 4 tiles (512 tokens).
  - x tile [128t, 512d] f32 is cast to bf16 (DVE), PE-transposed into
    xT [128d, (c t)] (bf16 identity comes pre-cast from host).
  - MM1: hT[h,t] = W1_chunk.T @ xT (feature-major hidden), fused
    bias+gelu on ScalarE into actT (bf16).
  - LoRA: g[t,24] = gelu(xT.T @ wdown_all) (PE, N=24 passes), scaled
    per-expert by routing weights into a packed g2 group tile
    [128t, 4*32] whose col 32i+24 is 1.0 (carries b2); one DMA XBAR
    transpose per group gives gt [128, 128] with tile i at partition
    base 32i.
  - MM2 per token tile: 16 h-chunk passes (lhsT = actT slices) into
    PSUM; LoRA-up closers are row-tiled (tile_position=(32i,0), K=25:
    24 LoRA rows + a ones-row adding b2) and run packed in pairs.
  - PSUM drains to SBUF on GpSimd (otherwise idle), DVE handles the
    x cast/pack path and routing/LoRA element-wise work.
  - routing weights wts[e,t] = sum_k probs[t,k]*(idx[t,k]==e) computed on
    device in a small DVE prologue over all tokens at once.
  - matmuls in bf16 (full PE rate, FWL weight loads); fp32 PSUM accum.
"""

import os
import numpy as np

P = 128
D = 512
DH = 2048
E = 3
R = 8
ER = E * R  # 24
ERB = ER + 1  # 25: + ones row for b2
NH = DH // P  # 16
NDC = D // P  # 4
N_CORES = 8
T_FULL = 64 * 28 * 28  # 50176
TC = T_FULL // N_CORES  # 6272
GROUP_TILES = 4

_CACHE = {}


def _build(tc_tokens, use_gelu=True):
    import concourse.bacc as bacc
    import concourse.tile as tile
    import concourse.mybir as mybir
    from contextlib import ExitStack

    f32 = mybir.dt.float32
    bf16 = mybir.dt.bfloat16
    i32 = mybir.dt.int32
    AF = mybir.ActivationFunctionType
    act_fn = AF.Gelu if use_gelu else AF.Relu
    OP = mybir.AluOpType

    nt = tc_tokens // P  # token tiles
    assert tc_tokens % P == 0

    nc = bacc.Bacc("TRN2", target_bir_lowering=False, debug=False,
                   num_devices=N_CORES)

    x = nc.dram_tensor("x", [tc_tokens, D], f32, kind="ExternalInput").ap()
    # pre-packed weights (host-side layout prep, bf16):
    w1p = nc.dram_tensor("w1p", [P, NDC * DH], bf16, kind="ExternalInput").ap()
    w2p = nc.dram_tensor("w2p", [P, NH * D], bf16, kind="ExternalInput").ap()
    wdp = nc.dram_tensor("wdp", [P, NDC * ER], bf16, kind="ExternalInput").ap()
    wup = nc.dram_tensor("wup", [P, D], bf16, kind="ExternalInput").ap()
    b1p = nc.dram_tensor("b1p", [P, NH], f32, kind="ExternalInput").ap()
    identp = nc.dram_tensor("identp", [P, P], bf16, kind="ExternalInput").ap()
    tkp = nc.dram_tensor("tkp", [tc_tokens, 2], f32, kind="ExternalInput").ap()
    tki = nc.dram_tensor("tki", [tc_tokens, 4], i32, kind="ExternalInput").ap()
    out = nc.dram_tensor("out", [tc_tokens, D], f32, kind="ExternalOutput").ap()

    with tile.TileContext(nc) as tc, ExitStack() as ctx:
        cons = ctx.enter_context(tc.tile_pool(name="cons", bufs=1))
        xin = ctx.enter_context(tc.tile_pool(name="xin", bufs=12))
        xbp = ctx.enter_context(tc.tile_pool(name="xbp", bufs=6))
        xtp = ctx.enter_context(tc.tile_pool(name="xtp", bufs=3))
        actp = ctx.enter_context(tc.tile_pool(name="actp", bufs=2))
        outp = ctx.enter_context(tc.tile_pool(name="outp", bufs=6))
        gp = ctx.enter_context(tc.tile_pool(name="gp", bufs=5))
        ps_xt = ctx.enter_context(tc.tile_pool(name="ps_xt", bufs=2, space="PSUM"))
        ps_h = ctx.enter_context(tc.tile_pool(name="ps_h", bufs=3, space="PSUM"))
        ps_o = ctx.enter_context(tc.tile_pool(name="ps_o", bufs=3, space="PSUM"))

        # ---- identity (bf16 from host; gates the very first transposes) ----
        ident_b = cons.tile([P, P], bf16)
        nc.sync.dma_start(ident_b[:], identp)

        # ---- prefetch x tiles of groups 0-1 ahead of the weight DMAs ----
        prefetched = {}
        for i in range(min(2 * GROUP_TILES, nt)):
            x_t = xin.tile([P, D], f32, tag="x_t", name=f"x_t{i}")
            nc.sync.dma_start(x_t[:], x[i * P:(i + 1) * P, :])
            prefetched[i] = x_t

        # ---- weights/routing: direct DMA into SBUF (chunked, bf16) ----
        W1s = cons.tile([P, NDC * DH], bf16)
        for c in range(NDC):
            nc.sync.dma_start(W1s[:, c * DH:(c + 1) * DH],
                              w1p[:, c * DH:(c + 1) * DH])
        b1_sb = cons.tile([P, NH], f32)
        nc.sync.dma_start(b1_sb[:], b1p)
        wdr = cons.tile([P, NDC * ER], bf16)
        nc.sync.dma_start(wdr[:], wdp)
        wur = cons.tile([P, D], bf16)
        nc.sync.dma_start(wur[:], wup)
        tp_sb = cons.tile([P, nt * 2], f32)
        nc.sync.dma_start(tp_sb.rearrange("p (n k) -> p n k", k=2),
                          tkp.rearrange("(n p) k -> p n k", p=P))
        ti_sb = cons.tile([P, nt * 4], i32)
        nc.sync.dma_start(ti_sb.rearrange("p (n k) -> p n k", k=4),
                          tki.rearrange("(n p) k -> p n k", p=P))
        W2s = cons.tile([P, NH * D], bf16)
        for q in range(4):
            nc.sync.dma_start(W2s[:, q * 4 * D:(q + 1) * 4 * D],
                              w2p[:, q * 4 * D:(q + 1) * 4 * D])

        # warm the ScalarE activation table before the first real GELU
        warm = cons.tile([1, 1], f32)
        nc.scalar.activation(warm[:], b1_sb[0:1, 0:1], act_fn)

        # warm the PE HAM clock gate: self-overwriting dummy matmuls on
        # scratch keep the PE busy from ~2us so the real stream runs at
        # 2.4GHz from its first pass (and absorb DMA-startup jitter).
        wsrc = cons.tile([P, 512], bf16)
        nc.vector.memset(wsrc[:], 0.0)
        ps_w = ps_xt.tile([P, 512], f32, tag="ps_x", name="ps_warm",
                          padded_shape=[P, 512])
        for wi in range(24):
            nc.tensor.matmul(ps_w[:], wsrc[:, :P], wsrc[:],
                             start=True, stop=True, skip_group_check=True)
        wdrain = cons.tile([1, 1], f32)
        nc.vector.tensor_copy(wdrain[:], ps_w[0:1, 0:1])

        # ---- transpose path: cast to bf16, PE transpose, pack xT ----
        def emit_xpose_group(t0g, ngg, x_list):
            Gg = ngg * P
            xTt = xtp.tile([P, NDC * Gg], bf16, tag="xT", name=f"xT{t0g}")
            for i in range(ngg):
                xb = xbp.tile([P, D], bf16, tag="xb", name=f"xb{t0g + i}")
                nc.vector.tensor_copy(xb[:], x_list[i][:])
                px = ps_xt.tile([P, 512], bf16, tag="ps_x", name=f"ps_x{t0g + i}",
                                padded_shape=[P, 1024])
                for c in range(NDC):
                    nc.tensor.matmul(px[:, c * P:(c + 1) * P],
                                     xb[:, c * P:(c + 1) * P],
                                     ident_b[:],
                                     is_transpose=True,
                                     start=(c == 0), stop=(c == NDC - 1))
                nc.vector.tensor_copy(
                    xTt.rearrange("p (c g) -> p c g", c=NDC)[:, :, i * P:(i + 1) * P],
                    px.rearrange("p (c g) -> p c g", c=NDC))
            return xTt

        xT_pre = {}
        with tc.high_priority():
            for t0p in (0, GROUP_TILES):
                if t0p < nt:
                    ngp = min(GROUP_TILES, nt - t0p)
                    xs = [prefetched.pop(t0p + i) for i in range(ngp)]
                    xT_pre[t0p] = emit_xpose_group(t0p, ngp, xs)

        # ---- routing weights prologue: wts[tok_p, e*nt + tile] ----
        idxf = cons.tile([P, nt * 2], f32)
        iv = ti_sb.rearrange("p (n k two) -> p n k two", k=2, two=2)
        nc.vector.tensor_copy(
            idxf.rearrange("p (n k one) -> p n k one", k=2, one=1),
            iv[:, :, :, 0:1])
        wts = cons.tile([P, E * nt], f32)
        for e in range(E):
            eq = cons.tile([P, nt * 2], f32, tag="eq", name=f"eq{e}")
            nc.vector.tensor_scalar(eq[:], idxf[:], float(e), None,
                                    op0=OP.is_equal)
            nc.vector.tensor_tensor(eq[:], eq[:], tp_sb[:], op=OP.mult)
            nc.vector.tensor_reduce(wts[:, e * nt:(e + 1) * nt],
                                    eq.rearrange("p (n k) -> p n k", k=2),
                                    axis=mybir.AxisListType.X, op=OP.add)

        # ---- main loop over supergroups ----
        # PSUM->SBUF drains are emitted one group late so the DVE serves the
        # next group's cast/pack work first (drains have ps_o slack).
        pending_drains = []

        def flush_drains():
            while pending_drains:
                po, tt = pending_drains.pop(0)
                o_sb = outp.tile([P, D], f32, tag="o_sb", name=f"o_sb{tt}")
                nc.vector.tensor_copy(o_sb[:], po[:])
                nc.sync.dma_start(out[tt * P:(tt + 1) * P, :], o_sb[:])

        t0 = 0
        while t0 < nt:
            ng = min(GROUP_TILES, nt - t0)
            G = ng * P

            # prefetch the group after next so DMA stays ahead of compute
            for pf in range(t0 + 2 * GROUP_TILES, min(t0 + 3 * GROUP_TILES, nt)):
                if pf not in prefetched:
                    x_t = xin.tile([P, D], f32, tag="x_t", name=f"x_t{pf}")
                    nc.sync.dma_start(x_t[:], x[pf * P:(pf + 1) * P, :])
                    prefetched[pf] = x_t
            if t0 in xT_pre:
                xT = xT_pre.pop(t0)
            else:
                x_ts = []
                for i in range(ng):
                    tt = t0 + i
                    if tt in prefetched:
                        x_t = prefetched.pop(tt)
                    else:
                        x_t = xin.tile([P, D], f32, tag="x_t", name=f"x_t{tt}")
                        nc.sync.dma_start(x_t[:], x[tt * P:(tt + 1) * P, :])
                    x_ts.append(x_t)
                xT = emit_xpose_group(t0, ng, x_ts)

            flush_drains()

            # LoRA-down chain first: PE N=24 passes -> gelu -> scale ->
            # packed g2 group tile -> one DMA XBAR transpose (lots of
            # slack before the row-tiled LoRA-up closers need gt).
            g2g = gp.tile([P, P], bf16, tag="g2g", name=f"g2g{t0}")
            nc.vector.memset(g2g[:], 1.0)

            def emit_lora_down(i):
                tt = t0 + i
                pg = ps_xt.tile([P, ER], f32, tag="ps_x", name=f"ps_lg{tt}",
                                padded_shape=[P, 512])
                for c in range(NDC):
                    nc.tensor.matmul(
                        pg[:],
                        xT[:, c * G + i * P: c * G + (i + 1) * P],
                        wdr[:, c * ER:(c + 1) * ER],
                        start=(c == 0), stop=(c == NDC - 1))
                g_sb = gp.tile([P, ER], f32, tag="g_sb", name=f"g_sb{tt}")
                nc.scalar.activation(g_sb[:], pg[:], act_fn)
                for e in range(E):
                    nc.vector.tensor_scalar(
                        g2g[:, i * 32 + e * R: i * 32 + (e + 1) * R],
                        g_sb[:, e * R:(e + 1) * R],
                        wts[:, e * nt + tt: e * nt + tt + 1], None, op0=OP.mult)

            # MM1 + bias + gelu -> actT; LoRA-down passes interleave into
            # the h stream (h=1..4) to pad the ps_h double-buffer cadence.
            actT = actp.tile([P, NH * G], bf16, tag="actT", name=f"actT{t0}")
            for h in range(NH):
                ph = ps_h.tile([P, 512], f32, tag="ps_hT", name=f"ps_hT{t0}_{h}")
                for c in range(NDC):
                    nc.tensor.matmul(
                        ph[:, :G],
                        W1s[:, c * DH + h * P: c * DH + (h + 1) * P],
                        xT[:, c * G:(c + 1) * G],
                        start=(c == 0), stop=(c == NDC - 1))
                nc.scalar.activation(actT[:, h * G:(h + 1) * G], ph[:, :G],
                                     act_fn, bias=b1_sb[:, h:h + 1], scale=1.0)
                if 1 <= h <= ng:
                    emit_lora_down(h - 1)
            gt = gp.tile([P, P], bf16, tag="gt", name=f"gt{t0}")
            nc.sync.dma_start(gt[:], g2g[:], transpose=True)

            # MM2: h-chunk passes per token tile; row-tiled LoRA-up + b2
            # closers packed in pairs (ps_o has 3 banks).
            for half in range(0, ng, 2):
                hi = min(half + 2, ng)
                pos = []
                for i in range(half, hi):
                    tt = t0 + i
                    po = ps_o.tile([P, D], f32, tag="ps_out", name=f"ps_out{tt}")
                    pos.append(po)
                    for h in range(NH):
                        nc.tensor.matmul(
                            po[:],
                            actT[:, h * G + i * P: h * G + (i + 1) * P],
                            W2s[:, h * D:(h + 1) * D],
                            start=(h == 0), stop=False)
                for j, i in enumerate(range(half, hi)):
                    nc.tensor.matmul(pos[j][:],
                                     gt[i * 32:i * 32 + ERB, :],
                                     wur[i * 32:i * 32 + ERB, :],
                                     start=False, stop=True,
                                     tile_position=(i * 32, 0))
                for j, i in enumerate(range(half, hi)):
                    pending_drains.append((pos[j], t0 + i))

            t0 += ng
        flush_drains()

    nc.compile()
    return nc


def _get_nc():
    key = ("full", TC)
    if key not in _CACHE:
        _CACHE[key] = _build(TC, use_gelu=True)
    return _CACHE[key]


def _make_in_maps(inputs, tc_tokens=TC, n_cores=N_CORES):
    import ml_dtypes
    bf = ml_dtypes.bfloat16

    x = np.ascontiguousarray(inputs["x"], dtype=np.float32)
    T = x.size // D
    x_flat = x.reshape(T, D)
    W1 = np.asarray(inputs["W1"], dtype=np.float32)
    W2 = np.asarray(inputs["W2"], dtype=np.float32)
    b1 = np.asarray(inputs["b1"], dtype=np.float32)
    b2 = np.asarray(inputs["b2"], dtype=np.float32)

    # w1p[p, c*DH + :] = W1[c*P + p, :]
    w1p = np.ascontiguousarray(
        W1.reshape(NDC, P, DH).transpose(1, 0, 2).reshape(P, NDC * DH)).astype(bf)
    # w2p[p, h*D + :] = W2[h*P + p, :]
    w2p = np.ascontiguousarray(
        W2.reshape(NH, P, D).transpose(1, 0, 2).reshape(P, NH * D)).astype(bf)
    wdn = np.asarray(inputs["w_down"], dtype=np.float32).transpose(1, 0, 2).reshape(D, ER)
    wdp = np.ascontiguousarray(
        wdn.reshape(NDC, P, ER).transpose(1, 0, 2).reshape(P, NDC * ER)).astype(bf)
    # wup: 4 replicas at partition base 32i; rows 0-23 = w_up, row 24 = b2
    wu = np.asarray(inputs["w_up"], dtype=np.float32).reshape(ER, D)
    wup = np.zeros((P, D), dtype=np.float32)
    for i in range(4):
        wup[i * 32:i * 32 + ER, :] = wu
        wup[i * 32 + ER, :] = b2
    wup = np.ascontiguousarray(wup).astype(bf)
    b1p = np.ascontiguousarray(b1.reshape(NH, P).T)
    identp = np.eye(P, dtype=np.float32).astype(bf)

    tkp = np.ascontiguousarray(inputs["topk_probs"], dtype=np.float32)
    tki_in = np.asarray(inputs["topk_indices"])
    tki = np.zeros((T, 4), dtype=np.int32)
    tki[:, 0] = tki_in[:, 0]
    tki[:, 2] = tki_in[:, 1]

    in_maps = []
    for c in range(n_cores):
        sl = slice(c * tc_tokens, (c + 1) * tc_tokens)
        in_maps.append(dict(
            x=np.ascontiguousarray(x_flat[sl]), w1p=w1p, w2p=w2p,
            wdp=wdp, wup=wup, b1p=b1p, identp=identp,
            tkp=np.ascontiguousarray(tkp[sl]),
            tki=np.ascontiguousarray(tki[sl])))
    return in_maps


def _ensure_ntff_hook():
    """Register the axon NTFF profile hook if the image's antenv lacks it."""
    import sys
    import types
    try:
        from antenv.axon_hooks import get_axon_ntff_profile_hook  # noqa: F401
        return True
    except ImportError:
        pass
    try:
        from trn_agent_boot.trn_boot import _ntff_profile_via_ctypes
        mod = types.ModuleType("antenv.axon_hooks")
        _hook = [None]
        mod.set_axon_ntff_profile_hook = lambda h: _hook.__setitem__(0, h)
        mod.get_axon_ntff_profile_hook = lambda: _hook[0]
        sys.modules["antenv.axon_hooks"] = mod
        import antenv
        antenv.axon_hooks = mod
        mod.set_axon_ntff_profile_hook(
            _ntff_profile_via_ctypes("/opt/axon/libaxon_pjrt.so"))
        return True
    except Exception:
        return False


def kernel(**inputs):
    from concourse.bass_utils import run_bass_kernel_spmd

    nc = _get_nc()
    in_maps = _make_in_maps(inputs)
    trace = bool(int(os.environ.get("KERNEL_TRACE", "0")))
    if trace and not _ensure_ntff_hook():
        trace = False
    res = run_bass_kernel_spmd(nc, in_maps, list(range(N_CORES)), trace=trace)
    if trace:
        _CACHE["last_result"] = res
    out = np.concatenate([res.results[i]["out"] for i in range(N_CORES)], axis=0)
    return out.reshape(np.asarray(inputs["x"]).shape).astype(np.float32)


# revision 11
# speedup vs baseline: 1.0090x; 1.0090x over previous
"""Trainium2 Bass kernel: ConvNeXt MLP + parallel MoE-LoRA (data-parallel over tokens).

Math per token t (D=512, Dh=2048, E=3 experts, r=8, top-k=2):
    base = gelu(x @ W1 + b1) @ W2 + b2
    g_e  = gelu(x @ w_down[e]) * wts[e, t]          (wts from top-k routing)
    out  = base + sum_e g_e @ w_up[e]

Strategy (per NeuronCore, 8 cores data-parallel on the token dim):
  - weights arrive pre-packed bf16 from host (chunked layouts ready for
    direct DMA into SBUF) -- no on-device weight staging/casting.
  - tokens tiled 128 at a time; supergroups of 4 tiles (512 tokens).
  - x tile [128t, 512d] f32 is cast to bf16 (DVE), PE-transposed into
    xT [128d, (c t)] (bf16 identity comes pre-cast from host).
  - MM1: hT[h,t] = W1_chunk.T @ xT (feature-major hidden), fused
    bias+gelu on ScalarE into actT (bf16).
  - LoRA: g[t,24] = gelu(xT.T @ wdown_all) (PE, N=24 passes), scaled
    per-expert by routing weights into a packed g2 group tile
    [128t, 4*32] whose col 32i+24 is 1.0 (carries b2); one DMA XBAR
    transpose per group gives gt [128, 128] with tile i at partition
    base 32i.
  - MM2 per token tile: 16 h-chunk passes (lhsT = actT slices) into
    PSUM; LoRA-up closers are row-tiled (tile_position=(32i,0), K=25:
    24 LoRA rows + a ones-row adding b2) and run packed in pairs.
  - PSUM->SBUF output drains run on DVE, emitted one group late so the
    next group's cast/pack work wins the DVE queue (ps_o has slack).
  - routing weights wts[e,t] = sum_k probs[t,k]*(idx[t,k]==e) computed on
    device in a small DVE prologue over all tokens at once.
  - matmuls in bf16 (full PE rate, FWL weight loads); fp32 PSUM accum.
"""

import os
import numpy as np

P = 128
D = 512
DH = 2048
E = 3
R = 8
ER = E * R  # 24
ERB = ER + 1  # 25: + ones row for b2
NH = DH // P  # 16
NDC = D // P  # 4
N_CORES = 8
T_FULL = 64 * 28 * 28  # 50176
TC = T_FULL // N_CORES  # 6272
GROUP_TILES = 4

_CACHE = {}


def _build(tc_tokens, use_gelu=True):
    import concourse.bacc as bacc
    import concourse.tile as tile
    import concourse.mybir as mybir
    from contextlib import ExitStack

    f32 = mybir.dt.float32
    bf16 = mybir.dt.bfloat16
    i32 = mybir.dt.int32
    AF = mybir.ActivationFunctionType
    act_fn = AF.Gelu if use_gelu else AF.Relu
    OP = mybir.AluOpType

    nt = tc_tokens // P  # token tiles
    assert tc_tokens % P == 0

    nc = bacc.Bacc("TRN2", target_bir_lowering=False, debug=False,
                   num_devices=N_CORES)

    x = nc.dram_tensor("x", [tc_tokens, D], f32, kind="ExternalInput").ap()
    # pre-packed weights (host-side layout prep, bf16):
    w1p = nc.dram_tensor("w1p", [P, NDC * DH], bf16, kind="ExternalInput").ap()
    w2p = nc.dram_tensor("w2p", [P, NH * D], bf16, kind="ExternalInput").ap()
    wdp = nc.dram_tensor("wdp", [P, NDC * ER], bf16, kind="ExternalInput").ap()
    wup = nc.dram_tensor("wup", [P, D], bf16, kind="ExternalInput").ap()
    b1p = nc.dram_tensor("b1p", [P, NH], f32, kind="ExternalInput").ap()
    identp = nc.dram_tensor("identp", [P, P], bf16, kind="ExternalInput").ap()
    tkp = nc.dram_tensor("tkp", [tc_tokens, 2], f32, kind="ExternalInput").ap()
    tki = nc.dram_tensor("tki", [tc_tokens, 4], i32, kind="ExternalInput").ap()
    out = nc.dram_tensor("out", [tc_tokens, D], f32, kind="ExternalOutput").ap()

    with tile.TileContext(nc) as tc, ExitStack() as ctx:
        cons = ctx.enter_context(tc.tile_pool(name="cons", bufs=1))
        xin = ctx.enter_context(tc.tile_pool(name="xin", bufs=12))
        xbp = ctx.enter_context(tc.tile_pool(name="xbp", bufs=6))
        xtp = ctx.enter_context(tc.tile_pool(name="xtp", bufs=3))
        actp = ctx.enter_context(tc.tile_pool(name="actp", bufs=2))
        outp = ctx.enter_context(tc.tile_pool(name="outp", bufs=6))
        gp = ctx.enter_context(tc.tile_pool(name="gp", bufs=5))
        ps_xt = ctx.enter_context(tc.tile_pool(name="ps_xt", bufs=2, space="PSUM"))
        ps_h = ctx.enter_context(tc.tile_pool(name="ps_h", bufs=2, space="PSUM"))
        ps_o = ctx.enter_context(tc.tile_pool(name="ps_o", bufs=3, space="PSUM"))
        ps_g = ctx.enter_context(tc.tile_pool(name="ps_g", bufs=1, space="PSUM"))

        # ---- identity (bf16 from host; gates the very first transposes) ----
        ident_b = cons.tile([P, P], bf16)
        nc.sync.dma_start(ident_b[:], identp)

        # ---- prefetch x tiles of groups 0-1 ahead of the weight DMAs ----
        prefetched = {}
        for i in range(min(2 * GROUP_TILES, nt)):
            x_t = xin.tile([P, D], f32, tag="x_t", name=f"x_t{i}")
            nc.sync.dma_start(x_t[:], x[i * P:(i + 1) * P, :])
            prefetched[i] = x_t

        # ---- weights/routing: direct DMA into SBUF (chunked, bf16) ----
        W1s = cons.tile([P, NDC * DH], bf16)
        for c in range(NDC):
            nc.sync.dma_start(W1s[:, c * DH:(c + 1) * DH],
                              w1p[:, c * DH:(c + 1) * DH])
        b1_sb = cons.tile([P, NH], f32)
        nc.sync.dma_start(b1_sb[:], b1p)
        wdr = cons.tile([P, NDC * ER], bf16)
        nc.sync.dma_start(wdr[:], wdp)
        wur = cons.tile([P, D], bf16)
        nc.sync.dma_start(wur[:], wup)
        tp_sb = cons.tile([P, nt * 2], f32)
        nc.sync.dma_start(tp_sb.rearrange("p (n k) -> p n k", k=2),
                          tkp.rearrange("(n p) k -> p n k", p=P))
        ti_sb = cons.tile([P, nt * 4], i32)
        nc.sync.dma_start(ti_sb.rearrange("p (n k) -> p n k", k=4),
                          tki.rearrange("(n p) k -> p n k", p=P))
        W2s = cons.tile([P, NH * D], bf16)
        for q in range(4):
            nc.sync.dma_start(W2s[:, q * 4 * D:(q + 1) * 4 * D],
                              w2p[:, q * 4 * D:(q + 1) * 4 * D])

        # warm the ScalarE activation table before the first real GELU
        warm = cons.tile([1, 1], f32)
        nc.scalar.activation(warm[:], b1_sb[0:1, 0:1], act_fn)

        # warm the PE HAM clock gate: self-overwriting dummy matmuls on
        # scratch keep the PE busy from ~2us so the real stream runs at
        # 2.4GHz from its first pass (and absorb DMA-startup jitter).
        wsrc = cons.tile([P, 512], bf16)
        nc.vector.memset(wsrc[:], 0.0)
        ps_w = ps_g.tile([P, 512], f32, tag="ps_lg", name="ps_warm")
        for wi in range(24):
            nc.tensor.matmul(ps_w[:], wsrc[:, :P], wsrc[:],
                             start=True, stop=True, skip_group_check=True)
        wdrain = cons.tile([1, 1], f32)
        nc.vector.tensor_copy(wdrain[:], ps_w[0:1, 0:1])

        # ---- transpose path: cast to bf16, PE transpose, pack xT ----
        def emit_xpose_group(t0g, ngg, x_list):
            Gg = ngg * P
            xTt = xtp.tile([P, NDC * Gg], bf16, tag="xT", name=f"xT{t0g}")
            for i in range(ngg):
                xb = xbp.tile([P, D], bf16, tag="xb", name=f"xb{t0g + i}")
                nc.vector.tensor_copy(xb[:], x_list[i][:])
                px = ps_xt.tile([P, 512], bf16, tag="ps_x", name=f"ps_x{t0g + i}")
                for c in range(NDC):
                    nc.tensor.matmul(px[:, c * P:(c + 1) * P],
                                     xb[:, c * P:(c + 1) * P],
                                     ident_b[:],
                                     is_transpose=True,
                                     start=(c == 0), stop=(c == NDC - 1))
                nc.vector.tensor_copy(
                    xTt.rearrange("p (c g) -> p c g", c=NDC)[:, :, i * P:(i + 1) * P],
                    px.rearrange("p (c g) -> p c g", c=NDC))
            return xTt

        xT_pre = {}
        with tc.high_priority():
            for t0p in (0, GROUP_TILES):
                if t0p < nt:
                    ngp = min(GROUP_TILES, nt - t0p)
                    xs = [prefetched.pop(t0p + i) for i in range(ngp)]
                    xT_pre[t0p] = emit_xpose_group(t0p, ngp, xs)

        # ---- routing weights prologue: wts[tok_p, e*nt + tile] ----
        idxf = cons.tile([P, nt * 2], f32)
        iv = ti_sb.rearrange("p (n k two) -> p n k two", k=2, two=2)
        nc.vector.tensor_copy(
            idxf.rearrange("p (n k one) -> p n k one", k=2, one=1),
            iv[:, :, :, 0:1])
        wts = cons.tile([P, E * nt], f32)
        for e in range(E):
            eq = cons.tile([P, nt * 2], f32, tag="eq", name=f"eq{e}")
            nc.vector.tensor_scalar(eq[:], idxf[:], float(e), None,
                                    op0=OP.is_equal)
            nc.vector.tensor_tensor(eq[:], eq[:], tp_sb[:], op=OP.mult)
            nc.vector.tensor_reduce(wts[:, e * nt:(e + 1) * nt],
                                    eq.rearrange("p (n k) -> p n k", k=2),
                                    axis=mybir.AxisListType.X, op=OP.add)

        # ---- main loop over supergroups ----
        # PSUM->SBUF drains are emitted one group late so the DVE serves the
        # next group's cast/pack work first (drains have ps_o slack).
        pending_drains = []

        def flush_drains():
            while pending_drains:
                po, tt = pending_drains.pop(0)
                o_sb = outp.tile([P, D], f32, tag="o_sb", name=f"o_sb{tt}")
                nc.vector.tensor_copy(o_sb[:], po[:])
                nc.sync.dma_start(out[tt * P:(tt + 1) * P, :], o_sb[:])

        t0 = 0
        while t0 < nt:
            ng = min(GROUP_TILES, nt - t0)
            G = ng * P

            # prefetch the group after next so DMA stays ahead of compute
            for pf in range(t0 + 2 * GROUP_TILES, min(t0 + 3 * GROUP_TILES, nt)):
                if pf not in prefetched:
                    x_t = xin.tile([P, D], f32, tag="x_t", name=f"x_t{pf}")
                    nc.sync.dma_start(x_t[:], x[pf * P:(pf + 1) * P, :])
                    prefetched[pf] = x_t
            if t0 in xT_pre:
                xT = xT_pre.pop(t0)
            else:
                x_ts = []
                for i in range(ng):
                    tt = t0 + i
                    if tt in prefetched:
                        x_t = prefetched.pop(tt)
                    else:
                        x_t = xin.tile([P, D], f32, tag="x_t", name=f"x_t{tt}")
                        nc.sync.dma_start(x_t[:], x[tt * P:(tt + 1) * P, :])
                    x_ts.append(x_t)
                xT = emit_xpose_group(t0, ng, x_ts)

            flush_drains()

            # LoRA-down chain first: PE N=24 passes -> gelu -> scale ->
            # packed g2 group tile -> one DMA XBAR transpose (lots of
            # slack before the row-tiled LoRA-up closers need gt).
            g2g = gp.tile([P, P], bf16, tag="g2g", name=f"g2g{t0}")
            nc.vector.memset(g2g[:], 1.0)

            def emit_lora_down(i):
                tt = t0 + i
                pg = ps_g.tile([P, ER], f32, tag="ps_lg", name=f"ps_lg{tt}")
                for c in range(NDC):
                    nc.tensor.matmul(
                        pg[:],
                        xT[:, c * G + i * P: c * G + (i + 1) * P],
                        wdr[:, c * ER:(c + 1) * ER],
                        start=(c == 0), stop=(c == NDC - 1))
                g_sb = gp.tile([P, ER], f32, tag="g_sb", name=f"g_sb{tt}")
                nc.scalar.activation(g_sb[:], pg[:], act_fn)
                for e in range(E):
                    nc.vector.tensor_scalar(
                        g2g[:, i * 32 + e * R: i * 32 + (e + 1) * R],
                        g_sb[:, e * R:(e + 1) * R],
                        wts[:, e * nt + tt: e * nt + tt + 1], None, op0=OP.mult)

            # MM1 + bias + gelu -> actT; LoRA-down passes interleave into
            # the h stream (h=1..4) to pad the ps_h double-buffer cadence.
            actT = actp.tile([P, NH * G], bf16, tag="actT", name=f"actT{t0}")
            for h in range(NH):
                ph = ps_h.tile([P, 512], f32, tag="ps_hT", name=f"ps_hT{t0}_{h}")
                for c in range(NDC):
                    nc.tensor.matmul(
                        ph[:, :G],
                        W1s[:, c * DH + h * P: c * DH + (h + 1) * P],
                        xT[:, c * G:(c + 1) * G],
                        start=(c == 0), stop=(c == NDC - 1))
                nc.scalar.activation(actT[:, h * G:(h + 1) * G], ph[:, :G],
                                     act_fn, bias=b1_sb[:, h:h + 1], scale=1.0)
                if 1 <= h <= ng:
                    emit_lora_down(h - 1)
            gt = gp.tile([P, P], bf16, tag="gt", name=f"gt{t0}")
            nc.sync.dma_start(gt[:], g2g[:], transpose=True)

            # MM2: h-chunk passes per token tile; row-tiled LoRA-up + b2
            # closers packed in pairs (ps_o has 3 banks).
            for half in range(0, ng, 2):
                hi = min(half + 2, ng)
                pos = []
                for i in range(half, hi):
                    tt = t0 + i
                    po = ps_o.tile([P, D], f32, tag="ps_out", name=f"ps_out{tt}")
                    pos.append(po)
                    for h in range(NH):
                        nc.tensor.matmul(
                            po[:],
                            actT[:, h * G + i * P: h * G + (i + 1) * P],
                            W2s[:, h * D:(h + 1) * D],
                            start=(h == 0), stop=False)
                for j, i in enumerate(range(half, hi)):
                    nc.tensor.matmul(pos[j][:],
                                     gt[i * 32:i * 32 + ERB, :],
                                     wur[i * 32:i * 32 + ERB, :],
                                     start=False, stop=True,
                                     tile_position=(i * 32, 0))
                for j, i in enumerate(range(half, hi)):
                    pending_drains.append((pos[j], t0 + i))

            t0 += ng
        flush_drains()

    nc.compile()
    return nc


def _get_nc():
    key = ("full", TC)
    if key not in _CACHE:
        _CACHE[key] = _build(TC, use_gelu=True)
    return _CACHE[key]


def _make_in_maps(inputs, tc_tokens=TC, n_cores=N_CORES):
    import ml_dtypes
    bf = ml_dtypes.bfloat16

    x = np.ascontiguousarray(inputs["x"], dtype=np.float32)
    T = x.size // D
    x_flat = x.reshape(T, D)
    W1 = np.asarray(inputs["W1"], dtype=np.float32)
    W2 = np.asarray(inputs["W2"], dtype=np.float32)
    b1 = np.asarray(inputs["b1"], dtype=np.float32)
    b2 = np.asarray(inputs["b2"], dtype=np.float32)

    # w1p[p, c*DH + :] = W1[c*P + p, :]
    w1p = np.ascontiguousarray(
        W1.reshape(NDC, P, DH).transpose(1, 0, 2).reshape(P, NDC * DH)).astype(bf)
    # w2p[p, h*D + :] = W2[h*P + p, :]
    w2p = np.ascontiguousarray(
        W2.reshape(NH, P, D).transpose(1, 0, 2).reshape(P, NH * D)).astype(bf)
    wdn = np.asarray(inputs["w_down"], dtype=np.float32).transpose(1, 0, 2).reshape(D, ER)
    wdp = np.ascontiguousarray(
        wdn.reshape(NDC, P, ER).transpose(1, 0, 2).reshape(P, NDC * ER)).astype(bf)
    # wup: 4 replicas at partition base 32i; rows 0-23 = w_up, row 24 = b2
    wu = np.asarray(inputs["w_up"], dtype=np.float32).reshape(ER, D)
    wup = np.zeros((P, D), dtype=np.float32)
    for i in range(4):
        wup[i * 32:i * 32 + ER, :] = wu
        wup[i * 32 + ER, :] = b2
    wup = np.ascontiguousarray(wup).astype(bf)
    b1p = np.ascontiguousarray(b1.reshape(NH, P).T)
    identp = np.eye(P, dtype=np.float32).astype(bf)

    tkp = np.ascontiguousarray(inputs["topk_probs"], dtype=np.float32)
    tki_in = np.asarray(inputs["topk_indices"])
    tki = np.zeros((T, 4), dtype=np.int32)
    tki[:, 0] = tki_in[:, 0]
    tki[:, 2] = tki_in[:, 1]

    in_maps = []
    for c in range(n_cores):
        sl = slice(c * tc_tokens, (c + 1) * tc_tokens)
        in_maps.append(dict(
            x=np.ascontiguousarray(x_flat[sl]), w1p=w1p, w2p=w2p,
            wdp=wdp, wup=wup, b1p=b1p, identp=identp,
            tkp=np.ascontiguousarray(tkp[sl]),
            tki=np.ascontiguousarray(tki[sl])))
    return in_maps


def _ensure_ntff_hook():
    """Register the axon NTFF profile hook if the image's antenv lacks it."""
    import sys
    import types
    try:
        from antenv.axon_hooks import get_axon_ntff_profile_hook  # noqa: F401
        return True
    except ImportError:
        pass
    try:
        from trn_agent_boot.trn_boot import _ntff_profile_via_ctypes
        mod = types.ModuleType("antenv.axon_hooks")
        _hook = [None]
        mod.set_axon_ntff_profile_hook = lambda h: _hook.__setitem__(0, h)
        mod.get_axon_ntff_profile_hook = lambda: _hook[0]
        sys.modules["antenv.axon_hooks"] = mod
        import antenv
        antenv.axon_hooks = mod
        mod.set_axon_ntff_profile_hook(
            _ntff_profile_via_ctypes("/opt/axon/libaxon_pjrt.so"))
        return True
    except Exception:
        return False


def kernel(**inputs):
    from concourse.bass_utils import run_bass_kernel_spmd

    nc = _get_nc()
    in_maps = _make_in_maps(inputs)
    trace = bool(int(os.environ.get("KERNEL_TRACE", "0")))
    if trace and not _ensure_ntff_hook():
        trace = False
    res = run_bass_kernel_spmd(nc, in_maps, list(range(N_CORES)), trace=trace)
    if trace:
        _CACHE["last_result"] = res
    out = np.concatenate([res.results[i]["out"] for i in range(N_CORES)], axis=0)
    return out.reshape(np.asarray(inputs["x"]).shape).astype(np.float32)
